# revision 7
# baseline (speedup 1.0000x reference)
"""DGCNN-style edge-conv block (KNN graph + dense conv stack) on 8 trn2 cores.

Optimized rev 2. Key changes vs the 319.7us baseline:

  top-16   = packed score+index values instead of full-width DVE passes.
             ACT evacuates score PSUM with bias (-xx_i + 1023.5 + 2^17);
             the fp32 store at magnitude 2^17 rounds the score to a 2^-6
             grid. A Pool-engine scalar_tensor_tensor subtracts 2^17 and
             adds iota (c mod 256)*2^-14 into the 8 freed mantissa bits.
             DVE then does 8 chunk-max8s (256 wide), a 64-wide top-16
             stage (max8 / max_index / match_replace / max8 / max_index),
             and recovers global indices arithmetically from the packed
             winners: idx = (pos>>3)<<8 | (bits & 0xFF). This removes the
             two full-width MaxIndex and one full-width MatchReplace
             passes per tile (3 x 2194ns -> ~900ns of 64-wide ops).
  scores   = f32r matmuls (1 cyc/col vs 4 for fp32 on PE).
  gather   = P table in fp16 (half DMA traffic; fp16 transposes on PE
             run at 1 cyc/row vs 2).
  edge MLP = all fp16 activations/weights; DVE k-max trees hit the
             2x_1p 16-bit mode. T-bias add (sa) runs on the Pool engine.
             Partition-fold copies (hi) moved from ACT to DMA.

Engine budget estimate per core: DVE ~110us, ACT ~95us, Pool ~92us,
PE ~60us -> total ~130-150us vs 319.7us baseline.
"""

import numpy as np

import bass_rust
import concourse.bass as bass
import concourse.bass_isa as bass_isa
import concourse.mybir as mybir
from concourse.bass_types import AP
from concourse.tile import TileContext
from concourse.bass_utils import run_bass_kernel_spmd

F32 = mybir.dt.float32
F32R = mybir.dt.float32r
F16 = mybir.dt.float16
U16 = mybir.dt.uint16
I16 = mybir.dt.int16
I32 = mybir.dt.int32

B, C, N, K, G = 8, 64, 2048, 16, 64
NT = 16          # 128-row tiles
NSC = 4          # super-chunks
NBL = 4          # nblocks per super-chunk
NCH = 8          # score chunks per tile (of 256 cols each)
RELU = mybir.ActivationFunctionType.Relu
COPY = mybir.ActivationFunctionType.Copy
IDENT = mybir.ActivationFunctionType.Identity
SQUARE = mybir.ActivationFunctionType.Square
ADD = mybir.AluOpType.add
MAX = mybir.AluOpType.max
BAND = mybir.AluOpType.bitwise_and
SHR = mybir.AluOpType.logical_shift_right
SHL = mybir.AluOpType.logical_shift_left

QBIAS = 131072.0          # 2^17: forces fp32 rounding of scores to 2^-6
SHIFT = 1023.5            # places t = score + SHIFT in (600, 1024)

_nop_ctr = [0]


def _split_all_waits(nc, max_waits=1):
    # This walrus build rejects >1 sync-wait on several CTRL structs; hoist
    # extras onto single-wait NOPs placed just before the instruction.
    for fn in nc.m.functions:
        for bb in fn.blocks:
            out = []
            for ins in bb.instructions:
                si = ins.sync_info
                if si is not None and si.on_wait is not None and len(si.on_wait) > max_waits:
                    waits = list(si.on_wait)
                    for w in waits[:-max_waits]:
                        _nop_ctr[0] += 1
                        nop = mybir.InstNoOp(name=f"waitnop-{_nop_ctr[0]}", ins=[], outs=[])
                        nop.engine = ins.engine
                        nop.sync_info = bass_rust.SyncInfo(on_wait=[w], on_update=[])
                        out.append(nop)
                        nc.register_instruction(nop, overwrite=True)
                    si.on_wait = waits[-max_waits:]
                out.append(ins)
            bb.instructions = out


def _insert_gpsimd_library_load(nc, lib_index=3):
    # InstDMAGatherAnt needs the 'mlp' GPSIMD ucode library; raw Bass+Tile
    # skips Bacc's insert_library_loads, so prepend the reload by hand.
    ins = bass_isa.InstPseudoReloadLibraryIndex(
        name="libload-manual", ins=[], outs=[], lib_index=lib_index
    )
    ins.engine = mybir.EngineType.Pool
    nc.register_instruction(ins, overwrite=True)
    bb0 = nc.m.functions[0].blocks[0]
    bb0.instructions = [ins] + list(bb0.instructions)
    mybir.codegen_inst_isa_subclasses(nc)


def build():
    nc = bass.Bass("TRN2", debug=False, num_devices=8)

    x_in = nc.dram_tensor("x", [C, N], F32R, kind="ExternalInput")
    WLTP = nc.dram_tensor("WLTP", [64, 64], F32R, kind="ExternalInput")    # W1a.T
    WLT = nc.dram_tensor("WLT", [65, 64], F32R, kind="ExternalInput")      # [(W1b-W1a).T; b1]
    WLR = nc.dram_tensor("WLR", [65, 64], F32R, kind="ExternalInput")      # [W2b.T; b2]
    WLS = nc.dram_tensor("WLS", [65, 64], F32R, kind="ExternalInput")      # [W3b.T; b3]
    W2BLK = nc.dram_tensor("W2BLK", [128, 128], F16, kind="ExternalInput")
    W3ABLK = nc.dram_tensor("W3ABLK", [128, 128], F16, kind="ExternalInput")
    W3CBLK = nc.dram_tensor("W3CBLK", [128, 128], F16, kind="ExternalInput")
    EYE16 = nc.dram_tensor("EYE16", [128, 128], F16, kind="ExternalInput")
    EYE32 = nc.dram_tensor("EYE32", [128, 128], F32, kind="ExternalInput")
    IOTA14 = nc.dram_tensor("IOTA14", [128, N], F16, kind="ExternalInput")
    Y = nc.dram_tensor("y", [C + 3 * G, N], F32, kind="ExternalOutput")

    PT_D = nc.dram_tensor("PT_D", [N, 2 * C], F16, kind="Internal")
    IDXD = nc.dram_tensor("IDXD", [N * K], I16, kind="Internal")

    with TileContext(nc) as tc:
        with tc.tile_pool(name="const", bufs=1) as cp, \
             tc.tile_pool(name="work", bufs=2) as wp, \
             tc.tile_pool(name="chunk", bufs=2) as kp, \
             tc.tile_pool(name="gat", bufs=2) as gp, \
             tc.tile_pool(name="ps2", bufs=2, space="PSUM") as pps2, \
             tc.tile_pool(name="ps1", bufs=2, space="PSUM") as pps1:

            # ---------------- setup ----------------
            X65 = cp.tile([65, N], F32R)
            RHSB = cp.tile([128, N], F32R)
            LHSB = cp.tile([128, N], F32R)
            X2 = cp.tile([64, N], F32)
            NXXC = cp.tile([128, NT], F32)
            PC = cp.tile([64, N], F16)
            TSTK = cp.tile([128, N], F16)
            RSTK = cp.tile([128, N], F16)
            SCt = cp.tile([64, N], F32)
            PTS = cp.tile([128, NT * 128], F16)
            IDXALL = cp.tile([128, NT * K], U16)
            EYEt = cp.tile([128, 128], F16)
            EYE32t = cp.tile([128, 128], F32)
            IOTAt = cp.tile([128, N], F16)
            ONES64 = cp.tile([64, 1], F32)
            CBIAS = cp.tile([128, 1], F32)
            wltp = cp.tile([64, 64], F32R)
            wlt = cp.tile([65, 64], F32R)
            wlr = cp.tile([65, 64], F32R)
            wls = cp.tile([65, 64], F32R)
            w2b = cp.tile([128, 128], F16)
            w3a = cp.tile([128, 128], F16)
            w3c = cp.tile([128, 128], F16)

            X65f = X65.bitcast(F32)
            RHSBf = RHSB.bitcast(F32)
            LHSBf = LHSB.bitcast(F32)

            for cu0 in range(4):
                sl0 = slice(cu0 * 512, (cu0 + 1) * 512)
                nc.sync.dma_start(out=X65[0:64, sl0], in_=x_in[:, sl0])
                nc.sync.dma_start(out=RHSB[0:64, sl0], in_=x_in[:, sl0])
            nc.sync.dma_start(out=EYEt[:, :], in_=EYE16[:, :])
            nc.sync.dma_start(out=EYE32t[:, :], in_=EYE32[:, :])
            nc.sync.dma_start(out=IOTAt[:, :], in_=IOTA14[:, :])
            nc.sync.dma_start(out=wltp[:, :], in_=WLTP[:, :])
            nc.sync.dma_start(out=wlt[:, :], in_=WLT[:, :])
            nc.sync.dma_start(out=wlr[:, :], in_=WLR[:, :])
            nc.sync.dma_start(out=wls[:, :], in_=WLS[:, :])
            nc.sync.dma_start(out=w2b[:, :], in_=W2BLK[:, :])
            nc.sync.dma_start(out=w3a[:, :], in_=W3ABLK[:, :])
            nc.sync.dma_start(out=w3c[:, :], in_=W3CBLK[:, :])
            # f32r memset fails ISA encoding; write constants via ACT
            nc.scalar.activation(X65[64:65, :], X65f[0:1, :], COPY,
                                 scale=0.0, bias=1.0)
            nc.scalar.activation(LHSB[64:128, :], X65f[0:64, :], COPY,
                                 scale=0.0, bias=-1.0)
            nc.gpsimd.memset(ONES64[:, :], 1.0)
            nc.gpsimd.memset(CBIAS[:, :], SHIFT + QBIAS)
            nc.gpsimd.memset(PTS[:, :], 0.0)

            for cu in range(4):
                slc = slice(cu * 512, (cu + 1) * 512)
                nc.scalar.activation(X2[:, slc], X65f[0:64, slc], SQUARE)
                nc.scalar.activation(RHSB[64:128, slc], X2[:, slc], COPY)
                nc.scalar.activation(LHSB[0:64, slc], X65f[0:64, slc], COPY, scale=2.0)
                ps_xx = pps1.tile([128, 4], F32, tag="u2")
                for j in range(4):
                    rt = cu * 4 + j
                    nc.tensor.matmul(ps_xx[:, j:j + 1],
                                     X2[:, rt * 128:(rt + 1) * 128],
                                     ONES64[:, :], start=True, stop=True)
                nc.scalar.activation(NXXC[:, cu * 4:(cu + 1) * 4], ps_xx[:, :], IDENT,
                                     scale=-1.0, bias=CBIAS[:, 0:1])

            def emit_ptrs_setup():
              # P only -- the gather table's sole dependency
              for u in range(4):
                sl = slice(u * 512, (u + 1) * 512)
                p1 = pps1.tile([64, 512], F32, tag="u2")
                nc.tensor.matmul(p1[:, :], wltp[:, :], RHSB[0:64, sl], start=True, stop=True)
                nc.scalar.activation(PC[:, sl], p1[:, :], COPY)

              # P^T table -> DRAM
              for rg in range(4):
                pt4 = pps1.tile([128, 4, 64], F16, tag="u2")
                for j in range(4):
                    rt = rg * 4 + j
                    nc.tensor.transpose(pt4[:, j, :], PC[:, rt * 128:(rt + 1) * 128],
                                        EYEt[0:64, 0:64])
                nc.scalar.activation(
                    PTS[:, rg * 512:(rg + 1) * 512]
                        .rearrange("p (a b) -> p a b", a=4)[:, :, 0:64],
                    pt4[:, :, :], COPY)
              nc.sync.dma_start(
                out=AP(PT_D, 0, [[128, 128], [16384, NT], [1, 128]]),
                in_=PTS[:, :].rearrange("p (a b) -> p a b", a=NT),
              )

            # x passthrough output rows 64:128
            nc.sync.dma_start(out=Y[64:128, :], in_=X65f[0:64, :])

            def emit_subgather(rt, PG):
                # per-tile: idx -> DRAM in gather-wrapped order -> 2048-idx
                # gather into this tile's 16 slots of the super-chunk PG.
                # addr = (node%16)*128 + k*8 + node//16; the write AP splits
                # the partition iteration (node = rhi*16 + pp) accordingly.
                nbl = rt % 4
                nc.sync.dma_start(
                    out=AP(IDXD, rt * 2048, [[1, 8], [128, 16], [8, K]]),
                    in_=IDXALL[:, rt * K:(rt + 1) * K].bitcast(I16),
                )
                idxt = gp.tile([128, 128], I16, tag="idxt", bufs=4)
                nc.sync.dma_start(out=idxt[0:16, :],
                                  in_=AP(IDXD, rt * 2048, [[128, 16], [1, 128]]))
                for g in (16, 32, 64):
                    nc.sync.dma_start(out=idxt[g:2 * g, :], in_=idxt[0:g, :])
                nc.gpsimd.dma_gather(
                    out_ap=PG[:, nbl * 16:(nbl + 1) * 16, :], in_ap=PT_D.ap(),
                    idxs_ap=idxt[:, :],
                    num_idxs=2048, num_idxs_reg=2048, elem_size=128,
                    single_packet=False,
                )

            PGs = {}
            # ---------------- scores + topk for all row tiles ----------------
            for rt in range(NT):
                S32 = wp.tile([128, N], F32, tag="scores")
                cand = wp.tile([128, 64], F32, tag="cand", bufs=3)
                for u in range(2):
                    pss = pps2.tile([128, 1024], F32, tag="score")
                    for h in range(2):
                        nc.tensor.matmul(pss[:, h * 512:(h + 1) * 512],
                                         LHSB[:, rt * 128:(rt + 1) * 128],
                                         RHSB[:, u * 1024 + h * 512:u * 1024 + (h + 1) * 512],
                                         start=True, stop=True)
                    half = slice(u * 1024, (u + 1) * 1024)
                    # fp32 store at ~2^17 rounds score to 2^-6 grid
                    nc.scalar.activation(S32[:, half], pss[:, :], IDENT,
                                         bias=NXXC[:, rt:rt + 1])
                    # unpack to [512,1024) binade + add per-column iota
                    nc.vector.scalar_tensor_tensor(
                        out=S32[:, half], in0=S32[:, half], scalar=-QBIAS,
                        in1=IOTAt[:, half], op0=ADD, op1=ADD)
                    # chunk max8s
                    for ch in range(4):
                        c = u * 4 + ch
                        nc.vector.max(out=cand[:, c * 8:(c + 1) * 8],
                                      in_=S32[:, c * 256:(c + 1) * 256])

                # 64-wide top-16 stage
                t16 = wp.tile([128, 16], F32, tag="t16", bufs=3)
                pos16 = wp.tile([128, 16], U16, tag="pos16", bufs=3)
                nc.vector.max(out=t16[:, 0:8], in_=cand[:, :])
                nc.vector.max_index(out=pos16[:, 0:8], in_max=t16[:, 0:8],
                                    in_values=cand[:, :])
                nc.vector.match_replace(out=cand[:, :], in_to_replace=t16[:, 0:8],
                                        in_values=cand[:, :], imm_value=-3.0e38)
                nc.vector.max(out=t16[:, 8:16], in_=cand[:, :])
                nc.vector.max_index(out=pos16[:, 8:16], in_max=t16[:, 8:16],
                                    in_values=cand[:, :])

                # idx = (pos>>3)<<8 | (valuebits & 0xFF); all-u16 bit ops
                base16 = wp.tile([128, 16], U16, tag="base16")
                loc16 = wp.tile([128, 16], U16, tag="loc16")
                tlow = t16[:, :].bitcast(U16).rearrange(
                    "p (a two) -> p a two", two=2)[:, :, 0]
                nc.vector.tensor_scalar(out=base16[:, :], in0=pos16[:, :],
                                        scalar1=3, scalar2=8, op0=SHR, op1=SHL)
                nc.vector.tensor_scalar(out=loc16[:, :], in0=tlow,
                                        scalar1=255, scalar2=None, op0=BAND)
                nc.vector.tensor_tensor(out=IDXALL[:, rt * K:(rt + 1) * K],
                                        in0=base16[:, :], in1=loc16[:, :], op=ADD)
                if rt == 0:
                    # the gather table must be written before any sub-gather
                    # is emitted (PT_D raw-AP reads are not dep-tracked)
                    emit_ptrs_setup()
                if rt == 2:
                    # T/R/S tables are first needed by the conv phase; defer
                    # their ACT evacs so early score evacs are not blocked
                    for u in range(4):
                        sl = slice(u * 512, (u + 1) * 512)
                        p2 = pps1.tile([64, 512], F32, tag="u2")
                        nc.tensor.matmul(p2[:, :], wlt[:, :], X65[:, sl], start=True, stop=True)
                        nc.scalar.activation(TSTK[0:64, sl], p2[:, :], COPY)
                        p3 = pps1.tile([64, 512], F32, tag="u2")
                        nc.tensor.matmul(p3[:, :], wlr[:, :], X65[:, sl], start=True, stop=True)
                        nc.scalar.activation(RSTK[0:64, sl], p3[:, :], COPY)
                        p4 = pps1.tile([64, 512], F32, tag="u2")
                        nc.tensor.matmul(p4[:, :], wls[:, :], X65[:, sl], start=True, stop=True)
                        nc.scalar.activation(SCt[:, sl], p4[:, :], COPY)
                    nc.sync.dma_start(out=TSTK[64:128, :], in_=TSTK[0:64, :])
                    nc.sync.dma_start(out=RSTK[64:128, :], in_=RSTK[0:64, :])
                if rt % 4 == 0:
                    PG_t = gp.tile([128, 64, 128], F16, tag="pg")
                    PGs[rt // 4] = PG_t
                emit_subgather(rt, PGs[rt // 4])

            # ---------------- per super-chunk ----------------
            for sc in range(NSC):
                PG = PGs[sc]
                AC = kp.tile([128, NBL, 8, 128], F16, tag="ac")
                B2C = kp.tile([128, NBL, 8, 128], F16, tag="b2c")
                C3C = kp.tile([128, NBL, 8, 128], F16, tag="c3c")

                for bl in range(NBL):
                    g = sc * NBL + bl
                    tb = TSTK[:, g * 128:(g + 1) * 128].unsqueeze(1).broadcast_to([128, 4, 128])
                    rb = RSTK[:, g * 128:(g + 1) * 128].unsqueeze(1).broadcast_to([128, 4, 128])
                    for q in range(2):
                        # transposes: 4 kp blocks -> psum (128, 512)
                        psa = pps2.tile([128, 512], F16, tag="a")
                        for kk in range(4):
                            kpi = q * 4 + kk
                            s0 = bl * 16 + 2 * kpi
                            nc.tensor.transpose(psa[0:64, kk * 128:(kk + 1) * 128],
                                                PG[:, s0, 0:64], EYEt[:, :])
                            nc.tensor.transpose(psa[64:128, kk * 128:(kk + 1) * 128],
                                                PG[:, s0 + 1, 0:64], EYEt[:, :])
                        sa = wp.tile([128, 512], F16, tag="sa", bufs=4)
                        nc.vector.tensor_tensor(out=sa[:, :], in0=psa[:, :],
                                                in1=tb, op=ADD)
                        nc.scalar.activation(AC[:, bl, 4 * q:4 * q + 4, :], sa[:, :], RELU)

                        # conv2 (+R folded in as an identity-matmul accumulate)
                        ps2t = pps1.tile([128, 512], F32, tag="u2")
                        nc.tensor.matmul(ps2t[:, :], w2b[:, :],
                                         AC[:, bl, 4 * q:4 * q + 4, :],
                                         start=True, stop=False,
                                         skip_group_check=True)
                        nc.tensor.matmul(ps2t[:, :].rearrange("p (a b) -> p a b", a=4),
                                         EYEt[:, :], rb,
                                         start=False, stop=True,
                                         skip_group_check=True)
                        nc.scalar.activation(B2C[:, bl, 4 * q:4 * q + 4, :], ps2t[:, :], RELU)

                        # conv3 (accumulate two matmuls)
                        ps3t = pps1.tile([128, 512], F32, tag="u2")
                        nc.tensor.matmul(ps3t[:, :], w3a[:, :],
                                         AC[:, bl, 4 * q:4 * q + 4, :],
                                         start=True, stop=False)
                        nc.tensor.matmul(ps3t[:, :], w3c[:, :],
                                         B2C[:, bl, 4 * q:4 * q + 4, :],
                                         start=False, stop=True)
                        nc.scalar.activation(C3C[:, bl, 4 * q:4 * q + 4, :], ps3t[:, :], COPY)

                # maxes over k (fp16 trees on DVE hit the 2x 16-bit mode),
                # then fold partition halves (DMA copy + DVE max) and DMA out.
                for (src, row0, add_s) in ((AC, 0, False), (B2C, 2 * G, False), (C3C, 3 * G, True)):
                    m1 = kp.tile([128, NBL, 4, 128], F16, tag="m1")
                    nc.vector.tensor_tensor(out=m1[:, :, :, :], in0=src[:, :, 0:4, :],
                                            in1=src[:, :, 4:8, :], op=MAX)
                    m2 = kp.tile([128, NBL, 2, 128], F16, tag="m2")
                    nc.vector.tensor_tensor(out=m2[:, :, :, :], in0=m1[:, :, 0:2, :],
                                            in1=m1[:, :, 2:4, :], op=MAX)
                    red = kp.tile([128, NBL, 128], F16, tag=f"red{row0}")
                    nc.vector.tensor_tensor(out=red[:, :, :], in0=m2[:, :, 0, :],
                                            in1=m2[:, :, 1, :], op=MAX)
                    hi = kp.tile([64, NBL * 128], F16, tag=f"hi{row0}")
                    nc.scalar.activation(
                        hi[:, :],
                        red[64:128, :, :].rearrange("p a n -> p (a n)"), COPY)
                    om = kp.tile([64, NBL * 128], F32, tag=f"om{row0}")
                    nc.vector.tensor_tensor(out=om[:, :],
                                            in0=red[0:64, :, :].rearrange("p a n -> p (a n)"),
                                            in1=hi[:, :], op=MAX)
                    if add_s:
                        nc.vector.tensor_tensor(out=om[:, :], in0=om[:, :],
                                                in1=SCt[:, sc * 512:(sc + 1) * 512],
                                                op=ADD)
                    nc.sync.dma_start(out=Y[row0:row0 + 64,
                                            sc * 512:(sc + 1) * 512],
                                      in_=om[:, :])

    _split_all_waits(nc)
    _insert_gpsimd_library_load(nc, 3)
    return nc


def _prep_weights(W1, b1, W2, b2, W3, b3):
    W1 = np.asarray(W1, np.float32); W2 = np.asarray(W2, np.float32)
    W3 = np.asarray(W3, np.float32)
    b1 = np.asarray(b1, np.float32); b2 = np.asarray(b2, np.float32)
    b3 = np.asarray(b3, np.float32)
    W1a, W1b = W1[:, :64], W1[:, 64:]
    W2a, W2b = W2[:, :64], W2[:, 64:]
    W3a, W3b, W3c = W3[:, :64], W3[:, 64:128], W3[:, 128:]

    def blk(w):
        z = np.zeros((128, 128), np.float16)
        z[0:64, 0:64] = w.T.astype(np.float16)
        z[64:128, 64:128] = w.T.astype(np.float16)
        return z

    iota = ((np.arange(N) % 256).astype(np.float32) * 2.0**-14).astype(np.float16)

    return {
        "WLTP": np.ascontiguousarray(W1a.T),
        "WLT": np.ascontiguousarray(np.vstack([(W1b - W1a).T, b1[None, :]])),
        "WLR": np.ascontiguousarray(np.vstack([W2b.T, b2[None, :]])),
        "WLS": np.ascontiguousarray(np.vstack([W3b.T, b3[None, :]])),
        "W2BLK": blk(W2a),
        "W3ABLK": blk(W3a),
        "W3CBLK": blk(W3c),
        "EYE16": np.eye(128, dtype=np.float16),
        "EYE32": np.eye(128, dtype=np.float32),
        "IOTA14": np.ascontiguousarray(np.broadcast_to(iota, (128, N))),
    }


_NC = None


def kernel(x, W1, b1, W2, b2, W3, b3):
    global _NC
    if _NC is None:
        _NC = build()
    x = np.asarray(x, np.float32)
    w = _prep_weights(W1, b1, W2, b2, W3, b3)
    in_maps = [{"x": np.ascontiguousarray(x[b]), **w} for b in range(B)]
    res = run_bass_kernel_spmd(_NC, in_maps, core_ids=list(range(B)))
    return np.stack([res.results[b]["y"] for b in range(B)], axis=0)
